# revision 45
# baseline (speedup 1.0000x reference)
"""CollisionLoss Trainium2 kernel (v7 — hardware-legal 4-engine pipeline).

Computes sum over (t, n) of the x/y AABB intersection area between the ego
(SDC) box at timestep t and ground-truth box n at timestep t, masked by the
per-timestep planning mask.

Math: for interval [lo, hi] (ego AABB on one axis) and corner coords x0..x3,
    inter_len = clamp(max_i x_i) - clamp(min_i x_i),  clamp(v) = min(max(v,lo),hi)
because clamping commutes with max/min (monotone) — no relu needed. The
max/min trees run directly on the raw DMA'd corners; only the two reduced
values per coord are clamped. area = len_x * len_y >= 0.

Engine split (every op here is legal on its engine per the walrus ISA check —
notably Pool only supports tensor_scalar, not tensor_tensor/STT):
  DVE : pairwise max/min trees, subtract, multiply (all tensor_tensor)
  Pool: dual-op clamps via tensor_scalar (per-partition ego bounds)
  Act : per-chunk accumulation (activation Copy with accum_out)
  SP  : all DMA
Partial sums land in acc[128, NCH] (f32), DMA'd out; host sums.

Sharding: future_gt_corners [T=256, N=16384, 4, 2] is sharded along N across
8 cores (2048 boxes/core). Host pre-transposes each core's slice to planar
[TBLK=2, plane=8, t=128, n=NL] bf16 (plane = coord*4 + corner) so all on-core
ops are packed 2-byte SBUF ops.
"""

import sys
from contextlib import ExitStack

import numpy as np

sys.path.insert(0, "/opt/trn_rl_repo")
sys.path.insert(0, "/opt/trn_rl_repo/concourse")

import concourse.bass as bass
import concourse.mybir as mybir

from concourse.bass_utils import run_bass_kernel_spmd

T = 256
N = 16384
NCORES = 8
NL = N // NCORES          # 2048 boxes per core
TBLK = T // 128           # 2 partition blocks
MAXB = 1024
DELTA = 0.5
WEIGHT = 1.0
EGO_W = 1.85 + DELTA
EGO_H = 4.084 + DELTA

F32 = mybir.dt.float32
BF16 = mybir.dt.bfloat16
Alu = mybir.AluOpType
ActF = mybir.ActivationFunctionType

# chunk schedule: (blk, box offset, B). Sums to NL per blk.
CHUNKS = [
    (0, 0, 256), (0, 256, 256), (0, 512, 256), (0, 768, 256),
    (0, 1024, 512), (0, 1536, 512),
    (1, 0, 512), (1, 512, 512), (1, 1024, 512), (1, 1536, 256), (1, 1792, 256),
]
NXTBUF = 4   # xt ring depth
NTAILSUB = 3  # last chunks subtract on DVE (skip the SWDGE drain at the end)

# consts cols per blk: ego AABB bounds
C_AXM = 0   # axm
C_AXX = 1   # axM
C_AYM = 2   # aym
C_AYX = 3   # ayM
NCC = 4


def build_kernel() -> bass.Bass:
    nc = bass.Bass(detect_race_conditions=False)
    x_d = nc.declare_dram_parameter("corners", [TBLK, 8, 128, NL], BF16,
                                    isOutput=False)
    c_d = nc.declare_dram_parameter("consts", [128, TBLK * NCC], F32,
                                    isOutput=False)
    NCH = len(CHUNKS)
    out_d = nc.declare_dram_parameter("partial", [128, NCH], F32, isOutput=True)

    LAST = NCH - 1
    with ExitStack() as ctx:
        consts = ctx.enter_context(nc.sbuf_tensor([128, TBLK * NCC], F32))
        acc = ctx.enter_context(nc.sbuf_tensor([128, NCH], F32))
        xts = [ctx.enter_context(nc.sbuf_tensor(f"xt{p}", [128, 8, MAXB], BF16))
               for p in range(NXTBUF)]
        s1Ms = [ctx.enter_context(
            nc.sbuf_tensor(f"s1M{p}", [128, 2, 2, MAXB], BF16)) for p in range(2)]
        s1Ns = [ctx.enter_context(
            nc.sbuf_tensor(f"s1N{p}", [128, 2, 2, MAXB], BF16)) for p in range(2)]
        MNs = [ctx.enter_context(
            nc.sbuf_tensor(f"MN{p}", [128, 2, 2, MAXB], BF16)) for p in range(4)]
        D = ctx.enter_context(nc.sbuf_tensor("D", [128, 2, MAXB], BF16))
        NNs = [ctx.enter_context(nc.sbuf_tensor(f"NN{p}", [128, 2, MAXB], BF16))
               for p in range(4)]
        us = [ctx.enter_context(nc.sbuf_tensor(f"u{p}", [128, MAXB], BF16))
              for p in range(3)]
        usc = ctx.enter_context(nc.sbuf_tensor("usc", [128, MAXB], BF16))

        csem = ctx.enter_context(nc.semaphore("csem"))
        dsem = [ctx.enter_context(nc.semaphore(f"xs{i}")) for i in range(NCH)]
        d2_sem = [ctx.enter_context(nc.semaphore(f"d2{i}")) for i in range(NCH)]
        po_sem = [ctx.enter_context(nc.semaphore(f"po{i}")) for i in range(NCH)]
        mu_sem = [ctx.enter_context(nc.semaphore(f"mu{i}")) for i in range(NCH)]
        ac_sem = [ctx.enter_context(nc.semaphore(f"ac{i}")) for i in range(NCH)]
        ng_sem = [ctx.enter_context(nc.semaphore(f"ng{i}")) for i in range(NCH)]
        gd_sem = [ctx.enter_context(nc.semaphore(f"gd{i}")) for i in range(NCH)]
        osem = ctx.enter_context(nc.semaphore("os"))
        block = ctx.enter_context(nc.Block())

        def cc(blk, col):  # [128,1] f32 const view
            return consts[:, blk * NCC + col : blk * NCC + col + 1]

        @block.sync
        def _(sp):
            for i, (blk, off, B) in enumerate(CHUNKS):
                if i == 1:
                    sp.dma_start(consts[:], c_d[:]).then_inc(csem, 16)
                p = i % NXTBUF
                src = x_d[blk][:, :, off : off + B]
                if i >= NXTBUF:
                    sp.wait_ge(d2_sem[i - NXTBUF], 1)
                sp.dma_start(
                    xts[p][:, 0:4, 0:B],
                    src[0:4].rearrange("l t b -> t l b"),
                ).then_inc(dsem[i], 16)
                sp.dma_start(
                    xts[p][:, 4:8, 0:B],
                    src[4:8].rearrange("l t b -> t l b"),
                ).then_inc(dsem[i], 16)
            sp.wait_ge(ac_sem[LAST - 2], 1)
            sp.wait_ge(ac_sem[LAST - 1], 1)
            with nc.allow_non_contiguous_dma(reason="tiny [128,1] f32 partial"):
                sp.dma_start(
                    out_d[:, 0:LAST], acc[:, 0:LAST]
                ).then_inc(osem, 16)
                sp.wait_ge(ac_sem[LAST], 1)
                sp.dma_start(
                    out_d[:, LAST : LAST + 1], acc[:, LAST : LAST + 1]
                ).then_inc(osem, 16)
            sp.wait_ge(osem, 32)

        @block.vector
        def _(v):
            def late(i):
                # multiply for chunk i. For mid chunks the difference D came
                # from the SWDGE add-DMA into NN; the tail chunk does its own
                # subtract + fused accum to keep the critical path short.
                _, _, B = CHUNKS[i]
                MN = MNs[i % 4][:]
                if i >= 3:
                    v.wait_ge(ac_sem[i - 3], 1)
                if i >= NCH - NTAILSUB:
                    v.tensor_tensor(
                        D[:, :, 0:B], MN[:, 0, :, 0:B], MN[:, 1, :, 0:B],
                        Alu.subtract,
                    )._wait_ge(po_sem[i], 2)
                    if i >= NCH - 2:
                        v.scalar_tensor_tensor(
                            us[i % 3][:, 0:B], D[:, 0, 0:B], 1.0, D[:, 1, 0:B],
                            Alu.bypass, Alu.mult,
                            accum_out=acc[:, i : i + 1],
                        ).then_inc(ac_sem[i], 1)
                    else:
                        v.tensor_tensor(
                            us[i % 3][:, 0:B], D[:, 0, 0:B], D[:, 1, 0:B],
                            Alu.mult,
                        ).then_inc(mu_sem[i], 1)
                else:
                    NN = NNs[i % 4][:]
                    v.tensor_tensor(
                        us[i % 3][:, 0:B], NN[:, 0, 0:B], NN[:, 1, 0:B],
                        Alu.mult,
                    )._wait_ge(gd_sem[i], 16).then_inc(mu_sem[i], 1)

            for i, (blk, off, B) in enumerate(CHUNKS):
                p = i % NXTBUF
                xv5 = xts[p][:].rearrange("t (c h l) b -> t c h l b", c=2, h=2)
                s1M = s1Ms[i % 2][:]
                s1N = s1Ns[i % 2][:]
                MN = MNs[i % 4][:]
                if 2 <= i:
                    # pipeline full: one wide inst per tree stage 1
                    v.tensor_tensor(
                        s1M[:, :, :, 0:B], xv5[:, :, 0, :, 0:B],
                        xv5[:, :, 1, :, 0:B], Alu.max,
                    )._wait_ge(dsem[i], 32)
                    v.tensor_tensor(
                        s1N[:, :, :, 0:B], xv5[:, :, 0, :, 0:B],
                        xv5[:, :, 1, :, 0:B], Alu.min,
                    )
                else:
                    v.tensor_tensor(
                        s1M[:, 0, :, 0:B], xv5[:, 0, 0, :, 0:B],
                        xv5[:, 0, 1, :, 0:B], Alu.max,
                    )._wait_ge(dsem[i], 16)
                    v.tensor_tensor(
                        s1N[:, 0, :, 0:B], xv5[:, 0, 0, :, 0:B],
                        xv5[:, 0, 1, :, 0:B], Alu.min,
                    )
                    v.tensor_tensor(
                        s1M[:, 1, :, 0:B], xv5[:, 1, 0, :, 0:B],
                        xv5[:, 1, 1, :, 0:B], Alu.max,
                    )._wait_ge(dsem[i], 32)
                    v.tensor_tensor(
                        s1N[:, 1, :, 0:B], xv5[:, 1, 0, :, 0:B],
                        xv5[:, 1, 1, :, 0:B], Alu.min,
                    )
                if i >= 4 and i - 4 < NCH - NTAILSUB:
                    v.wait_ge(gd_sem[i - 4], 16)
                v.tensor_tensor(
                    MN[:, 0, :, 0:B], s1M[:, :, 0, 0:B], s1M[:, :, 1, 0:B],
                    Alu.max,
                )
                v.tensor_tensor(
                    MN[:, 1, :, 0:B], s1N[:, :, 0, 0:B], s1N[:, :, 1, 0:B],
                    Alu.min,
                ).then_inc(d2_sem[i], 1)
                if i >= 3:
                    late(i - 3)
            late(LAST - 2)
            late(LAST - 1)
            late(LAST)

        @block.gpsimd
        def _(g):
            def dadd(i):
                # SWDGE DMA: NN (= -clamped-min, from Act) += clamped-max
                # so NN becomes the interval-length pair D
                _, _, B = CHUNKS[i]
                g.wait_ge(ng_sem[i], 1)
                g.dma_start(
                    NNs[i % 4][:, :, 0:B], MNs[i % 4][:, 0, :, 0:B],
                    accum_op=Alu.add,
                ).then_inc(gd_sem[i], 16)

            g.wait_ge(csem, 16)
            for i, (blk, off, B) in enumerate(CHUNKS):
                MN = MNs[i % 4][:]
                # dual-op clamp of [vM, vm] per coord against the ego bounds
                g.tensor_scalar(
                    MN[:, :, 0, 0:B], MN[:, :, 0, 0:B],
                    cc(blk, C_AXM), cc(blk, C_AXX), Alu.max, Alu.min,
                )._wait_ge(d2_sem[i], 1).then_inc(po_sem[i], 1)
                g.tensor_scalar(
                    MN[:, :, 1, 0:B], MN[:, :, 1, 0:B],
                    cc(blk, C_AYM), cc(blk, C_AYX), Alu.max, Alu.min,
                ).then_inc(po_sem[i], 1)
                if i >= 1 and i - 1 < NCH - NTAILSUB:
                    dadd(i - 1)

        @block.scalar
        def _(act):
            for i, (blk, off, B) in enumerate(CHUNKS):
                if i < NCH - NTAILSUB:
                    # NN = -clamped_min (reuse guarded by accum(i-1)'s mu wait:
                    # mult(i-2) precedes mult(i-1) on DVE)
                    act.activation(
                        NNs[i % 4][:, :, 0:B], MNs[i % 4][:, 1, :, 0:B],
                        ActF.Copy, scale=-1.0,
                    )._wait_ge(po_sem[i], 2).then_inc(ng_sem[i], 1)
                if i >= 3 and i - 3 != LAST:
                    k = i - 3
                    act.activation(
                        usc[:, 0 : CHUNKS[k][2]], us[k % 3][:, 0 : CHUNKS[k][2]],
                        ActF.Copy, accum_out=acc[:, k : k + 1],
                    )._wait_ge(mu_sem[k], 1).then_inc(ac_sem[k], 1)
            for k in (LAST - 3, LAST - 2):
                act.activation(
                    usc[:, 0 : CHUNKS[k][2]], us[k % 3][:, 0 : CHUNKS[k][2]],
                    ActF.Copy, accum_out=acc[:, k : k + 1],
                )._wait_ge(mu_sem[k], 1).then_inc(ac_sem[k], 1)

    return nc


_NC_CACHE: list = []


def _get_nc() -> bass.Bass:
    if not _NC_CACHE:
        _NC_CACHE.append(build_kernel())
    return _NC_CACHE[0]


def _host_aabb(sdc_traj_all, sdc_planning_gt, sdc_planning_gt_mask):
    """Ego box AABB per timestep, with mask folded in as degenerate boxes."""
    xy = np.asarray(sdc_traj_all, np.float32)[0, :, :2]          # [T, 2]
    yaw = np.asarray(sdc_planning_gt, np.float32)[0, :, 2]       # [T]
    base = np.array(
        [
            [EGO_W / 2, -EGO_H / 2],
            [EGO_W / 2, EGO_H / 2],
            [-EGO_W / 2, EGO_H / 2],
            [-EGO_W / 2, -EGO_H / 2],
        ],
        np.float32,
    )                                                            # [4, 2]
    c = np.cos(yaw, dtype=np.float32)
    s = np.sin(yaw, dtype=np.float32)
    rot = np.stack(
        [np.stack([c, s], -1), np.stack([-s, c], -1)], -2
    )                                                            # [T, 2, 2]
    corners = np.einsum("trc,kc->tkr", rot, base) + xy[:, None, :]  # [T, 4, 2]
    amax = corners.max(axis=1).astype(np.float32)                # [T, 2]
    amin = corners.min(axis=1).astype(np.float32)                # [T, 2]
    mask = np.asarray(sdc_planning_gt_mask)[0] != 0              # [T]
    amax = np.where(mask[:, None], amax, amin)                   # degenerate if masked
    return amin, amax


def _make_consts(amin, amax):
    """[128, TBLK*4] f32: per-blk cols axm, axM, aym, ayM."""
    axm = amin[:, 0].reshape(TBLK, 128)
    aym = amin[:, 1].reshape(TBLK, 128)
    axx = amax[:, 0].reshape(TBLK, 128)
    ayx = amax[:, 1].reshape(TBLK, 128)
    out = np.stack([axm, axx, aym, ayx], axis=-1)  # [TBLK, 128, 4]
    out = out.transpose(1, 0, 2).reshape(128, TBLK * NCC)
    return np.ascontiguousarray(out.astype(np.float32))


def _planar_corners(future_gt_corners):
    """[T, N, 4, 2] f32 -> per-core planar [TBLK, 8, 128, NL] bf16."""
    import ml_dtypes

    c = np.asarray(future_gt_corners).astype(ml_dtypes.bfloat16)
    # [blk, t, core, n, k, l]
    c6 = c.reshape(TBLK, 128, NCORES, NL, 4, 2)
    # -> [core, blk, l, k, t, n]
    c6 = c6.transpose(2, 0, 5, 4, 1, 3)
    # plane = l*4 + k
    return [np.ascontiguousarray(c6[core]).reshape(TBLK, 8, 128, NL)
            for core in range(NCORES)]


def kernel(sdc_traj_all, sdc_planning_gt, sdc_planning_gt_mask, future_gt_corners):
    amin, amax = _host_aabb(sdc_traj_all, sdc_planning_gt, sdc_planning_gt_mask)
    consts = _make_consts(amin, amax)
    planars = _planar_corners(future_gt_corners)

    in_maps = [
        {"corners": planars[core], "consts": consts} for core in range(NCORES)
    ]
    res = run_bass_kernel_spmd(_get_nc(), in_maps, list(range(NCORES)))
    total = np.float64(0.0)
    for core in range(NCORES):
        total += np.asarray(res.results[core]["partial"], np.float64).sum()
    return np.array([total * WEIGHT], np.float32)
